# revision 65
# baseline (speedup 1.0000x reference)
"""Trainium2 Bass kernel for nn_Attention_326417514823.

Per-batch computation (B=8, N=2048, D=256), one batch per NeuronCore:
    S = Q @ K.T / sqrt(D);  S[q,:] = -inf where mask[q]==0
    A = softmax(S, axis=0)  (normalize over q, per key column k)
    A[q,:] = 0 where mask[q]==0;  O = A @ V

Host-side restructuring (all exact):
  * q-compaction: masked q rows produce zero output rows and are excluded
    from the softmax normalizer, so the kernel only processes the nU
    unmasked q columns, padded to NQ (multiple of 64).  Pad columns carry
    Q=0 => exp(0)=1, removed from the normalizer via the host-computed
    count correction nmv = NQ - nU.  Host scatters rows back at the end.
  * fp8 residual split: X =~ X8 + dX8 (both e4m3) gives near-bf16-accurate
    matmuls out of fp8 DoubleRow instructions, which the PE runs at
    0.5 cycles/row (4x cheaper than bf16 per unit of contraction).

Device layout (k on partitions, q on the free axis; d-halves in dim1 for
DoubleRow):
    ST[k,q] = K8.Q8 + K8.dQ8 + dK8.Q8            (3 DR chains, PSUM f32)
    E16[k,q] = bf16(exp(ST/16))                  (Act engine, pure exp)
    c~[k] = sum_q E16 (bf16 DVE reduce at 4x);  rc = 1/(c~ - nmv)
    W16 = bf16(V16 * rc)
    OT[d,q] = sum_k W16^T E16                    (bf16 matmul chains)

Schedule: k-block-inner loop; scores double-buffered in 2x3 PSUM banks;
two [128,RES_Q] chain accumulators stay resident and accumulate
incrementally with a small k-block lag; the q>=RES_Q columns replay from
the persistent E16 tiles at the tail.  (GPSIMD compute is rejected by the
walrus engine checks, so everything elementwise lives on Act/DVE.)
"""

import numpy as np
import ml_dtypes

B, N, D = 8, 2048, 256
NCORES = 8
P = 128
KB = N // P          # 16 k-blocks
NPAIR = KB // 2      # 8 k-pairs for DoubleRow
MMW = 256            # max moving width per DR matmul (rhs free = 2*MMW)
RES_Q = 512          # resident chain width (1 PSUM bank of f32)
CHAIN_LAG = 3        # k-blocks of slack before resident chains consume W16

_cached = {}


def _q_chunks(nq, width):
    out = []
    q0 = 0
    while q0 < nq:
        out.append((q0, min(width, nq - q0)))
        q0 += width
    return out


def _build(NQ):
    import concourse.bacc as bacc
    import concourse.mybir as mybir
    import concourse.tile as tile

    f32 = mybir.dt.float32
    bf16 = mybir.dt.bfloat16
    fp8 = mybir.dt.float8e4
    MULT = mybir.AluOpType.mult
    SUB = mybir.AluOpType.subtract
    EXP = mybir.ActivationFunctionType.Exp
    DR = mybir.MatmulPerfMode.DoubleRow

    DQ = NQ - RES_Q            # deferred q width
    SCW = ((NQ * 4 + 6143) // 6144) * 1536  # score cols, 3-bank multiple

    nc = bacc.Bacc()
    kt8 = nc.dram_tensor("kt8", [P, 2, N], fp8, kind="ExternalInput")
    dkt8 = nc.dram_tensor("dkt8", [P, 2, N], fp8, kind="ExternalInput")
    qt8 = nc.dram_tensor("qt8", [P, 2, NQ], fp8, kind="ExternalInput")
    dqt8 = nc.dram_tensor("dqt8", [P, 2, NQ], fp8, kind="ExternalInput")
    vt = nc.dram_tensor("vt", [P, KB * D], bf16, kind="ExternalInput")
    nmv = nc.dram_tensor("nmv", [1, 1], f32, kind="ExternalInput")
    ot = nc.dram_tensor("ot", [2 * P, NQ], bf16, kind="ExternalOutput")

    with tile.TileContext(nc) as tc:
        with (
            tc.tile_pool(name="const", bufs=1) as constp,
            tc.tile_pool(name="e16p", bufs=1) as e16p,
            tc.tile_pool(name="redp", bufs=2) as redp,
            tc.tile_pool(name="wp", bufs=1) as wp,
            tc.tile_pool(name="outp", bufs=4) as outp,
            tc.tile_pool(name="psS", bufs=2, space="PSUM") as psS,
            tc.tile_pool(name="psC", bufs=1, space="PSUM") as psC,
        ):
            # ---- input staging -------------------------------------------
            # HWDGE prep is ~628ns/DMA and serializes globally, so inputs
            # travel in 8 DMAs ordered by first consumption: the kb0-3 K
            # chunks and the q operands first, everything else behind.
            kt_sb = constp.tile([P, 2, N], fp8, name="kt_sb")
            dkt_sb = constp.tile([P, 2, N], fp8, name="dkt_sb")
            qt_sb = constp.tile([P, 2, NQ], fp8, name="qt_sb")
            dqt_sb = constp.tile([P, 2, NQ], fp8, name="dqt_sb")
            v_sb = constp.tile([P, KB * D], bf16, name="v_sb")
            nmvb = constp.tile([P, 1], f32, name="nmvb")

            s03 = slice(0, 4 * P)
            s47 = slice(4 * P, 8 * P)
            s8f = slice(8 * P, N)
            # all input DMAs ride the SP ring: a dma_start holds the
            # issuing engine's sequencer until its HWDGE prep completes
            # (625ns each, globally serialized), and SP has nothing else
            nc.sync.dma_start(kt_sb[:, :, s03], kt8[:, :, s03])
            nc.sync.dma_start(qt_sb[:], qt8[:, :, :])
            nc.sync.dma_start(dqt_sb[:], dqt8[:, :, :])
            nc.sync.dma_start(dkt_sb[:, :, s03], dkt8[:, :, s03])
            nc.sync.dma_start(v_sb[:, 0:4 * D], vt[:, 0:4 * D])
            nc.sync.dma_start(nmvb[:], nmv[0:1, :].partition_broadcast(P))
            nc.sync.dma_start(kt_sb[:, :, s47], kt8[:, :, s47])
            nc.sync.dma_start(dkt_sb[:, :, s47], dkt8[:, :, s47])
            nc.sync.dma_start(v_sb[:, 4 * D:8 * D], vt[:, 4 * D:8 * D])
            nc.sync.dma_start(kt_sb[:, :, s8f], kt8[:, :, s8f])
            nc.sync.dma_start(dkt_sb[:, :, s8f], dkt8[:, :, s8f])
            nc.sync.dma_start(v_sb[:, 8 * D:], vt[:, 8 * D:])

            c16 = constp.tile([P, KB], f32, name="c16")
            cm16 = constp.tile([P, KB], f32, name="cm16")
            rc16 = constp.tile([P, KB], f32, name="rc16")

            # resident OT chain accumulators (q < RES_Q), 1 bank each
            ct = [psC.tile([P, RES_Q], f32, name=f"ct{dh}") for dh in range(2)]

            # warm the PE p-state during the input DMA wait; garbage lands in
            # ct[0] and is cleared by the chain's first start=True matmul
            zs = constp.tile([P, 2, P], fp8, name="zs")
            nc.vector.memset(zs[:], 0.0)
            for _ in range(20):
                nc.tensor.matmul(ct[0][:, 0:P], zs[:], zs[:],
                                 start=True, stop=True, perf_mode=DR)

            w16 = [None] * KB
            e16_hist = [None] * KB

            def chain_mm(kbl):
                for dh in range(2):
                    mi = nc.tensor.matmul(
                        ct[dh][:, 0:RES_Q],
                        w16[kbl][:, dh * P:(dh + 1) * P],
                        e16_hist[kbl][:, 0:RES_Q],
                        start=(kbl == 0),
                        stop=False,
                    )
                    # slack-filler: prefer mm1 when both are ready (but
                    # late chains stay prompt so the close isn't delayed)
                    mi.ins.bass_priority = (mi.ins.bass_priority or 0) + 200

            for kb in range(KB):
                g, kt = kb // 2, kb % 2

                # mm1: 3-term DR into a 3-bank score tile (term order matches
                # the input DMA arrival order)
                sc = psS.tile([P, SCW], f32, name="sc")
                terms = ((kt_sb, qt_sb), (kt_sb, dqt_sb), (dkt_sb, qt_sb))
                for q0, cw in _q_chunks(NQ, MMW):
                    for ti, (lt, rt) in enumerate(terms):
                        nc.tensor.matmul(
                            sc[:, q0:q0 + cw],
                            lt[:, :, kb * P:(kb + 1) * P],
                            rt[:, :, q0:q0 + cw],
                            start=(ti == 0),
                            stop=(ti == 2),
                            perf_mode=DR,
                        )

                e16 = e16p.tile([P, NQ], bf16, name=f"e16_{kb}")
                e16_hist[kb] = e16
                nc.scalar.activation(e16[:], sc[:, 0:NQ], EXP, scale=1.0 / 16.0)

                # resident chains (lagged) run in the exp shadow on the PE
                if kb >= CHAIN_LAG:
                    chain_mm(kb - CHAIN_LAG)

                # c~[kb]: bf16 reduce on DVE (2-byte SBUF operands hit the
                # 4x DVE mode); the copy output is a throwaway
                red = redp.tile([P, NQ], bf16, name="red")
                nc.vector.tensor_scalar(
                    red[:], e16[:], 1.0, 0.0, MULT,
                    mybir.AluOpType.add, accum_out=c16[:, kb:kb + 1])

                # c correction + W tile for this k-block
                s1 = slice(kb, kb + 1)
                nc.vector.tensor_scalar(
                    cm16[:, s1], c16[:, s1], nmvb[:], None, SUB)
                nc.vector.reciprocal(rc16[:, s1], cm16[:, s1])
                vsl = v_sb[:, kb * D:(kb + 1) * D]
                w16[kb] = wp.tile([P, D], bf16, name=f"w16_{kb}")
                nc.vector.tensor_scalar(
                    w16[kb][:], vsl, rc16[:, s1], None, MULT)


            # ---- tail ----------------------------------------------------
            def store(acc, dh, q0, cw):
                o_sb = outp.tile([P, RES_Q], bf16, name="o_sb")
                nc.scalar.mul(o_sb[:, 0:cw], acc[:, 0:cw], 1.0)
                nc.sync.dma_start(
                    ot[dh * P:(dh + 1) * P, q0:q0 + cw], o_sb[:, 0:cw])

            # deferred chunks replay from the persistent E16 tiles in banks
            # freed by the last two score buffers; a large prefix of each
            # chain is issued first (operands all ready, fills the PE while
            # the last exps/conversions drain), then the resident close +
            # stores, then the chain tails
            dq_chunks = _q_chunks(DQ, RES_Q)
            wide_cw = dq_chunks[0][1]
            dacc = [psS.tile([P, SCW], f32, name="sc") for _ in range(2)]

            def deferred_layout():
                # only the wide chunk lives in the score-pool banks; the
                # narrow remainder accumulates in the psC banks freed by the
                # resident stores, so the wide stores pipeline ahead of it
                for dh in range(2):
                    yield dacc[dh], 0, 0, wide_cw, dh

            SPLIT_KB = KB - 6
            for acc, aq0, q0, cw, dh in deferred_layout():
                for kb in range(SPLIT_KB):
                    nc.tensor.matmul(
                        acc[:, aq0:aq0 + cw],
                        w16[kb][:, dh * P:(dh + 1) * P],
                        e16_hist[kb][:, RES_Q + q0:RES_Q + q0 + cw],
                        start=(kb == 0),
                        stop=False,
                    )

            # resident chain tail closes next (it gates the resident stores)
            for kbl in range(KB - CHAIN_LAG, KB):
                for dh in range(2):
                    nc.tensor.matmul(
                        ct[dh][:, 0:RES_Q],
                        w16[kbl][:, dh * P:(dh + 1) * P],
                        e16_hist[kbl][:, 0:RES_Q],
                        start=False,
                        stop=(kbl == KB - 1),
                    )
            for dh in range(2):
                store(ct[dh], dh, 0, RES_Q)

            # one [P, 2, DQ] staging tile collects BOTH dh halves of the
            # deferred region, then a single DMA ships it (HWDGE preps are
            # the tail's serial resource)
            o_dall = outp.tile([P, 2, DQ], bf16, name="o_dall")
            o_dsb = [o_dall[:, dh, :] for dh in range(2)]
            for acc, aq0, q0, cw, dh in deferred_layout():
                for kb in range(SPLIT_KB, KB):
                    nc.tensor.matmul(
                        acc[:, aq0:aq0 + cw],
                        w16[kb][:, dh * P:(dh + 1) * P],
                        e16_hist[kb][:, RES_Q + q0:RES_Q + q0 + cw],
                        start=False,
                        stop=(kb == KB - 1),
                    )
                if dh == 0:
                    nc.scalar.mul(o_dsb[dh][:, 0:cw], acc[:, 0:cw], 1.0)
                else:
                    nc.vector.tensor_scalar(
                        o_dsb[dh][:, 0:cw], acc[:, 0:cw], 1.0, None, MULT)

            # narrow remainder: full chains in the freed resident banks
            if DQ > wide_cw:
                r0, rw = wide_cw, DQ - wide_cw
                for dh in range(2):
                    acc = psC.tile([P, RES_Q], f32, name=f"ct{dh}")
                    for kb in range(KB):
                        nc.tensor.matmul(
                            acc[:, 0:rw],
                            w16[kb][:, dh * P:(dh + 1) * P],
                            e16_hist[kb][:, RES_Q + r0:RES_Q + r0 + rw],
                            start=(kb == 0),
                            stop=(kb == KB - 1),
                        )
                    if dh == 0:
                        nc.scalar.mul(
                            o_dsb[dh][:, r0:r0 + rw], acc[:, 0:rw], 1.0)
                    else:
                        nc.vector.tensor_scalar(
                            o_dsb[dh][:, r0:r0 + rw], acc[:, 0:rw], 1.0,
                            None, MULT)
            # two DMAs: the wide part ships while the narrow remainder
            # chains/muls still close; only an 80B transfer remains at the end
            nc.sync.dma_start(
                ot[:, RES_Q:RES_Q + wide_cw].rearrange("(d p) q -> p d q", p=P),
                o_dall[:, :, 0:wide_cw])
            nc.sync.dma_start(
                ot[:, RES_Q + wide_cw:NQ].rearrange("(d p) q -> p d q", p=P),
                o_dall[:, :, wide_cw:DQ])

    nc.compile()
    return nc


def _get_nc(NQ=None):
    if NQ is None:
        if not _cached:
            raise RuntimeError("kernel not built yet")
        return next(iter(_cached.values()))
    if NQ not in _cached:
        _cached[NQ] = _build(NQ)
    return _cached[NQ]


def kernel(key, query, value, mask):
    from concourse.bass_utils import run_bass_kernel_spmd

    fp8 = ml_dtypes.float8_e4m3fn
    bf = ml_dtypes.bfloat16
    key = np.asarray(key, dtype=np.float32)
    query = np.asarray(query, dtype=np.float32)
    value = np.asarray(value, dtype=np.float32)
    mask = np.asarray(mask)

    idxs = [np.nonzero(mask[b, 0].astype(bool))[0] for b in range(B)]
    nU_max = max(len(ix) for ix in idxs)
    NQ = max(RES_Q + 64, ((nU_max + 7) // 8) * 8)
    nc = _get_nc(NQ)

    def split8(x):
        x8 = x.astype(fp8)
        dx8 = (x - x8.astype(np.float32)).astype(fp8)
        return x8, dx8

    in_maps = []
    for b in range(B):
        ix = idxs[b]
        nU = len(ix)
        qc = np.zeros((NQ, D), np.float32)
        qc[:nU] = query[b][ix]
        # [P, 2, cols] layouts: dim1 = d half (for QK) with d on partitions
        qt = np.ascontiguousarray(qc.T).reshape(2, P, NQ).transpose(1, 0, 2)
        kt = np.ascontiguousarray(key[b].T).reshape(2, P, N).transpose(1, 0, 2)
        qt8, dqt8 = split8(np.ascontiguousarray(qt))
        kt8, dkt8 = split8(np.ascontiguousarray(kt))
        vt = value[b].reshape(KB, P, D).transpose(1, 0, 2).reshape(P, KB * D)
        in_maps.append({
            "kt8": kt8, "dkt8": dkt8, "qt8": qt8, "dqt8": dqt8,
            "vt": np.ascontiguousarray(vt).astype(bf),
            "nmv": np.full((1, 1), float(NQ - nU), np.float32),
        })

    res = None
    for attempt in range(4):
        try:
            res = run_bass_kernel_spmd(nc, in_maps, core_ids=list(range(NCORES)))
            break
        except Exception:
            if attempt == 3:
                raise
            import time
            time.sleep(10 * (attempt + 1))
            try:
                import jax.extend.backend as _jb
                _jb.clear_backends()
                import jax
                jax.clear_caches()
            except Exception:
                pass

    out = np.zeros((B, N, D), np.float32)
    for b in range(B):
        ix = idxs[b]
        otb = res.results[b]["ot"].astype(np.float32)  # [2P, NQ]
        o = np.concatenate([otb[0:P], otb[P:2 * P]], axis=0).T  # [NQ, D]
        out[b][ix] = o[:len(ix)]
    return out
